# revision 10
# baseline (speedup 1.0000x reference)
"""Trainium2 Bass kernel for nn_DigitCap (CapsNet DigitCaps dynamic routing).

Computation (forward only, stop_gradient is a no-op for values):
    votes[b,i,o,a] = sum_k x[b,i,k] * W[i,k,(o,a)]          # B=16, I=2048, K=16, O=64, A=32
    logits = 0
    for it in 1..3:
        route = softmax_o(logits)
        pre[b,o,a] = sum_i route[b,i,o]*votes[b,i,o,a] + bias
        act = squash_a(pre)
        if it < 3: logits += sum_a votes[b,i,o,a]*act[b,o,a]
    return act

Distribution: shard I across 8 cores (256 capsules each).  Weights are read
once per core (16 MB fp16 slice), votes stay resident in SBUF in fp16.
The only cross-core coupling is the i-sum inside `pre`: two in-kernel
AllReduces of the 128 KB partial (iterations 1 and 2).  The final
iteration's partial is returned per-core and reduced + squashed on host.

On-device layout: j' = a*64 + o (a-outer) so that
  - softmax / squash reductions are clean free-dim group reductions
  - the distances a-reduction is a contiguous-halves TT-add tree
Partition layout of votes: p = b*8 + i_sub (b-outer) over groups g of 8
capsules; produced by a block-diagonal stationary x so each moving W column
feeds 128 useful MACs.
"""

import sys

sys.path.insert(0, "/opt/trn_rl_repo")

import numpy as np

import concourse.bass as bass
import concourse.bacc as bacc
import concourse.mybir as mybir
from concourse import tile
from concourse.bass_utils import run_bass_kernel_spmd

B = 16
I = 2048
K = 16  # input atoms
O = 64
A = 32  # output atoms
J = 2048  # O*A
NCORES = 8
ILOC = I // NCORES  # 256
G = ILOC // 8  # 32 groups of 8 capsules

F16 = mybir.dt.float16
F32 = mybir.dt.float32
AX = mybir.AxisListType
ALU = mybir.AluOpType
ACTFN = mybir.ActivationFunctionType


def _squash_host(pre):
    # pre: (B, A, O) in j' order (a outer, o inner); squash over a
    ns = np.sum(pre * pre, axis=1, keepdims=True)
    return pre * np.sqrt(ns) / (1.0 + ns)


def _device_softmax_route(nc, pools, logits_g, g):
    """softmax over o (innermost 64) of logits_g (128,64) fp32 -> route (128,64) f16."""
    expp, zsum, rcp, rpool = pools
    e = expp.tile([128, O], F16)
    nc.scalar.activation(e[:, :], logits_g, ACTFN.Exp)
    z = zsum.tile([128, 1], F32)
    nc.vector.tensor_reduce(z[:, :], e[:, :], axis=AX.X, op=ALU.add)
    zr = rcp.tile([128, 1], F32)
    nc.vector.reciprocal(zr[:, :], z[:, :])
    r = rpool.tile([128, O], F16)
    # route = exp * (1/Z)  on ACT (per-partition scalar scale)
    nc.scalar.activation(r[:, :], e[:, :], ACTFN.Copy, scale=zr[:, :])
    return r


def build_nc():
    nc = bacc.Bacc("TRN2", target_bir_lowering=False, debug=False, num_devices=NCORES)

    w_d = nc.declare_dram_parameter("w", [G, 128, J], F16, isOutput=False)
    xbd_d = nc.declare_dram_parameter("xbd", [128, G, 128], F16, isOutput=False)
    xdn_d = nc.declare_dram_parameter("xdn", [128, G, B], F16, isOutput=False)
    ones_d = nc.declare_dram_parameter("onesbd", [128, B], F16, isOutput=False)
    dup_d = nc.declare_dram_parameter("dup16", [B, 128], F16, isOutput=False)
    bias_d = nc.declare_dram_parameter("biasb", [B, J], F32, isOutput=False)
    out_d = nc.declare_dram_parameter("partial", [B, J], F32, isOutput=True)

    # collective bounce buffers (internal DRAM; outputs in Shared space)
    cc_in = [nc.dram_tensor(f"cc_in{t}", [B, J], F32) for t in range(2)]
    cc_out = [
        nc.dram_tensor(f"cc_out{t}", [B, J], F32, addr_space="Shared") for t in range(2)
    ]
    rg = [list(range(NCORES))]

    with tile.TileContext(nc) as tc:
        with (
            tc.tile_pool(name="const", bufs=1) as constp,
            tc.tile_pool(name="l1", bufs=1) as l1p,
            tc.tile_pool(name="wst", bufs=2) as wp,
            tc.tile_pool(name="mmps", bufs=4, space="PSUM") as mmps,
            tc.tile_pool(name="preps", bufs=1, space="PSUM") as preps,
            tc.tile_pool(name="dtmp", bufs=2) as dpool,
            tc.tile_pool(name="s1", bufs=2) as s1p,
            tc.tile_pool(name="s2", bufs=2) as s2p,
            tc.tile_pool(name="s3", bufs=2) as s3p,
            tc.tile_pool(name="s4", bufs=2) as s4p,
            tc.tile_pool(name="expp", bufs=3) as expp,
            tc.tile_pool(name="zsum", bufs=3) as zsum,
            tc.tile_pool(name="rcp", bufs=3) as rcp,
            tc.tile_pool(name="route", bufs=3) as routep,
            tc.tile_pool(name="small", bufs=1) as smallp,
            tc.tile_pool(name="actbx", bufs=1) as actbxp,
            tc.tile_pool(name="logits", bufs=1) as logitsp,
        ):
            softmax_pools = (expp, zsum, rcp, routep)

            # ---- constants ----
            xbd = constp.tile([128, G, 128], F16)
            nc.sync.dma_start(xbd[:, :, :], xbd_d[:, :, :])
            xdn = constp.tile([128, G, B], F16)
            nc.sync.dma_start(xdn[:, :, :], xdn_d[:, :, :])
            onesbd = constp.tile([128, B], F16)
            nc.sync.dma_start(onesbd[:, :], ones_d[:, :])
            dup16 = constp.tile([B, 128], F16)
            nc.sync.dma_start(dup16[:, :], dup_d[:, :])
            biasb = constp.tile([B, J], F32)
            nc.sync.dma_start(biasb[:, :], bias_d[:, :])

            L1 = l1p.tile([128, G, J], F16)  # resident votes, 16 MB
            logits = logitsp.tile([128, G, O], F16)

            # ================= P0: votes production + pre1 =================
            pre_ps = preps.tile([B, J], F32, tag="pre")
            for g in range(G):
                wt = wp.tile([128, J], F16)
                nc.sync.dma_start(wt[:, :], w_d[g, :, :])
                for c in range(4):
                    cs = slice(c * 512, (c + 1) * 512)
                    pm = mmps.tile([128, 512], F32)
                    nc.tensor.matmul(
                        pm[:, :], xbd[:, g, :], wt[:, cs], start=True, stop=True
                    )
                    # evacuate votes to resident f16, alternating engines
                    if c % 2 == 0:
                        nc.vector.tensor_copy(L1[:, g, cs], pm[:, :])
                    else:
                        nc.scalar.copy(L1[:, g, cs], pm[:, :])
                    # pre1 partial: uniform-route sum (xdn pre-scaled by 1/64)
                    nc.tensor.matmul(
                        pre_ps[:, cs],
                        xdn[:, g, :],
                        wt[:, cs],
                        start=(g == 0),
                        stop=(g == G - 1),
                    )

            # ================= iteration boundaries =================
            actbx = actbxp.tile([128, J], F16)

            def squash_to_actbx(cc_out_t):
                """DMA AR result in, + bias, squash, then broadcast to 128 partitions."""
                pre_sb = smallp.tile([B, J], F32, tag="pre_sb")
                nc.sync.dma_start(pre_sb[:, :], cc_out_t[:, :])
                nc.vector.tensor_add(pre_sb[:, :], pre_sb[:, :], biasb[:, :])
                sq = smallp.tile([B, J], F32, tag="preout")
                nc.scalar.activation(sq[:, :], pre_sb[:, :], ACTFN.Square)
                ns = smallp.tile([B, O], F32, tag="ns")
                nc.vector.tensor_reduce(
                    ns[:, :],
                    sq[:, :].rearrange("p (a o) -> p o a", a=A),
                    axis=AX.X,
                    op=ALU.add,
                )
                rt = smallp.tile([B, O], F32, tag="rt")
                nc.scalar.activation(rt[:, :], ns[:, :], ACTFN.Sqrt)
                # one Newton step: rt2 = 0.5*(rt + ns/rt)
                rci = smallp.tile([B, O], F32, tag="rci")
                nc.vector.reciprocal(rci[:, :], rt[:, :])
                nc.vector.tensor_mul(rci[:, :], rci[:, :], ns[:, :])
                nc.vector.tensor_add(rci[:, :], rci[:, :], rt[:, :])
                den = smallp.tile([B, O], F32, tag="den")
                nc.vector.tensor_scalar_add(den[:, :], ns[:, :], 1.0)
                nc.vector.reciprocal(den[:, :], den[:, :])
                s = smallp.tile([B, O], F32, tag="s")
                nc.vector.tensor_mul(s[:, :], den[:, :], rci[:, :])
                nc.vector.tensor_scalar_mul(s[:, :], s[:, :], 0.5)
                act16 = smallp.tile([B, J], F16, tag="act16")
                nc.vector.tensor_mul(
                    act16[:, :].rearrange("p (a o) -> p a o", a=A),
                    pre_sb[:, :].rearrange("p (a o) -> p a o", a=A),
                    s[:, :].rearrange("p (u o) -> p u o", u=1).broadcast_to((B, A, O)),
                )
                # broadcast act to (b,i)-partition layout via dup matmul
                for c in range(4):
                    cs = slice(c * 512, (c + 1) * 512)
                    pm = mmps.tile([128, 512], F32)
                    nc.tensor.matmul(
                        pm[:, :], dup16[:, :], act16[:, cs], start=True, stop=True
                    )
                    if c % 2 == 0:
                        nc.vector.tensor_copy(actbx[:, cs], pm[:, :])
                    else:
                        nc.scalar.copy(actbx[:, cs], pm[:, :])

            def iteration(t, pre_ps_prev, first_dist):
                """Evac prev pre partial -> AR -> squash -> distances+route+next pre."""
                pre_sb_out = smallp.tile([B, J], F32, tag="preout")
                nc.scalar.copy(pre_sb_out[:, :], pre_ps_prev[:, :])
                nc.sync.dma_start(cc_in[t][:, :], pre_sb_out[:, :])
                nc.gpsimd.collective_compute(
                    "AllReduce",
                    ALU.add,
                    replica_groups=rg,
                    ins=[cc_in[t][:, :]],
                    outs=[cc_out[t][:, :]],
                )
                squash_to_actbx(cc_out[t])

                pre_ps_next = preps.tile([B, J], F32, tag="pre")
                for g in range(G):
                    # ---- distances: dtmp = votes * actbx; tree-sum over a ----
                    dt = dpool.tile([128, J], F16, tag="dtmp")
                    nc.vector.tensor_mul(dt[:, :], L1[:, g, :], actbx[:, :])
                    s1 = s1p.tile([128, 1024], F16)
                    nc.vector.tensor_add(s1[:, :], dt[:, :1024], dt[:, 1024:])
                    s2 = s2p.tile([128, 512], F16)
                    nc.vector.tensor_add(s2[:, :], s1[:, :512], s1[:, 512:])
                    s3 = s3p.tile([128, 256], F16)
                    nc.vector.tensor_add(s3[:, :], s2[:, :256], s2[:, 256:])
                    s4 = s4p.tile([128, 128], F16)
                    nc.vector.tensor_add(s4[:, :], s3[:, :128], s3[:, 128:])
                    if first_dist:
                        nc.vector.tensor_add(
                            logits[:, g, :], s4[:, :64], s4[:, 64:]
                        )
                    else:
                        s5 = s4p.tile([128, 64], F16, tag="s5")
                        nc.vector.tensor_add(s5[:, :], s4[:, :64], s4[:, 64:])
                        nc.vector.tensor_add(logits[:, g, :], logits[:, g, :], s5[:, :])
                    # ---- route ----
                    r = _device_softmax_route(nc, softmax_pools, logits[:, g, :], g)
                    # ---- weighted votes + partition reduce ----
                    wv = dpool.tile([128, J], F16, tag="wv")
                    nc.vector.tensor_mul(
                        wv[:, :].rearrange("p (a o) -> p a o", a=A),
                        L1[:, g, :].rearrange("p (a o) -> p a o", a=A),
                        r[:, :].rearrange("p (u o) -> p u o", u=1).broadcast_to(
                            (128, A, O)
                        ),
                    )
                    for c in range(4):
                        cs = slice(c * 512, (c + 1) * 512)
                        nc.tensor.matmul(
                            pre_ps_next[:, cs],
                            onesbd[:, :],
                            wv[:, cs],
                            start=(g == 0),
                            stop=(g == G - 1),
                        )
                return pre_ps_next

            pre2_ps = iteration(0, pre_ps, first_dist=True)
            pre3_ps = iteration(1, pre2_ps, first_dist=False)

            out_sb = smallp.tile([B, J], F32, tag="preout")
            nc.scalar.copy(out_sb[:, :], pre3_ps[:, :])
            nc.sync.dma_start(out_d[:, :], out_sb[:, :])

    nc.finalize()
    return nc


_NC_CACHE = None


def _get_nc():
    global _NC_CACHE
    if _NC_CACHE is None:
        _NC_CACHE = build_nc()
    return _NC_CACHE


def prepare_inputs(x, weights):
    """Host-side sharding and layout prep. Returns list of per-core input dicts."""
    x = np.asarray(x, np.float32)[..., 0]  # (B, I, K)
    W = np.asarray(weights, np.float32)  # (I, K, J) with j = o*A + a

    # j' = a*64 + o  (a outer, o inner)
    Wp = (
        W.reshape(I, K, O, A).transpose(0, 1, 3, 2).reshape(I, K, J).astype(np.float16)
    )

    onesbd = np.zeros((128, B), np.float16)
    dup16 = np.zeros((B, 128), np.float16)
    for b in range(B):
        onesbd[b * 8 : (b + 1) * 8, b] = 1.0
        dup16[b, b * 8 : (b + 1) * 8] = 1.0

    in_maps = []
    for c in range(NCORES):
        xs = x[:, c * ILOC : (c + 1) * ILOC, :]  # (B, 256, K)
        # w: (G, 128, J) with row p = isub*16 + k
        wc = Wp[c * ILOC : (c + 1) * ILOC].reshape(G, 8 * K, J)
        # xbd: (128, G, 128): [isub*16+k, g, b*8+isub'] = x[b, 8g+isub, k] iff isub==isub'
        xbd = np.zeros((128, G, 128), np.float16)
        xdn = np.zeros((128, G, B), np.float16)
        xg = xs.reshape(B, G, 8, K)  # b, g, isub, k
        for isub in range(8):
            # rows isub*16 : isub*16+16, cols b*8+isub
            xbd[isub * K : (isub + 1) * K, :, isub::8] = xg[:, :, isub, :].transpose(
                2, 1, 0
            )
            xdn[isub * K : (isub + 1) * K, :, :] = (
                xg[:, :, isub, :].transpose(2, 1, 0) / 64.0
            )
        in_maps.append(
            {
                "w": np.ascontiguousarray(wc),
                "xbd": xbd,
                "xdn": xdn,
                "onesbd": onesbd,
                "dup16": dup16,
                "biasb": np.zeros((B, J), np.float32),  # placeholder, set below
            }
        )
    return in_maps


def kernel(x, weights, bias):
    bias = np.asarray(bias, np.float32)  # (O, A)
    in_maps = prepare_inputs(x, weights)
    biasb = np.broadcast_to(
        bias.T.reshape(1, J), (B, J)
    ).copy()  # j' = a*64+o -> bias.T is (A, O)
    for m in in_maps:
        m["biasb"] = biasb

    nc = _get_nc()
    res = run_bass_kernel_spmd(nc, in_maps, core_ids=list(range(NCORES)))
    partials = [res.results[c]["partial"] for c in range(NCORES)]

    total = np.sum(np.stack(partials, 0), axis=0, dtype=np.float64).astype(np.float32)
    pre3 = (total + biasb[0]).reshape(B, A, O)
    act = _squash_host(pre3)  # (B, A, O)
    return np.ascontiguousarray(act.transpose(0, 2, 1))  # (B, O, A)


# revision 20
# speedup vs baseline: 30.5771x; 30.5771x over previous
"""Trainium2 Bass kernel for nn_DigitCap (CapsNet DigitCaps dynamic routing).

Computation (forward only, stop_gradient is a no-op for values):
    votes[b,i,o,a] = sum_k x[b,i,k] * W[i,k,(o,a)]          # B=16, I=2048, K=16, O=64, A=32
    logits = 0
    for it in 1..3:
        route = softmax_o(logits)
        pre[b,o,a] = sum_i route[b,i,o]*votes[b,i,o,a] + bias
        act = squash_a(pre)
        if it < 3: logits += sum_a votes[b,i,o,a]*act[b,o,a]
    return act

Distribution: shard I across 8 cores (256 capsules each).  Weights are read
once per core (16 MB fp16 slice), votes stay resident in SBUF in fp16.
The only cross-core coupling is the i-sum inside `pre`: two in-kernel
AllReduces of the 128 KB partial (iterations 1 and 2).  The final
iteration's partial is returned per-core and reduced + squashed on host.

On-device layout: j' = a*64 + o (a-outer) so that
  - softmax / squash reductions are clean free-dim group reductions
  - the distances a-reduction is a contiguous-halves TT-add tree
Partition layout of votes: p = b*8 + i_sub (b-outer) over groups g of 8
capsules; produced by a block-diagonal stationary x so each moving W column
feeds 128 useful MACs.
"""

import sys

sys.path.insert(0, "/opt/trn_rl_repo")

import numpy as np

import concourse.bass as bass
import concourse.bacc as bacc
import concourse.mybir as mybir
from concourse import tile
from concourse.bass_utils import run_bass_kernel_spmd

B = 16
I = 2048
K = 16  # input atoms
O = 64
A = 32  # output atoms
J = 2048  # O*A
NCORES = 8
ILOC = I // NCORES  # 256
G = ILOC // 8  # 32 groups of 8 capsules

F16 = mybir.dt.float16
F32 = mybir.dt.float32
AX = mybir.AxisListType
ALU = mybir.AluOpType
ACTFN = mybir.ActivationFunctionType


def _squash_host(pre):
    # pre: (B, A, O) in j' order (a outer, o inner); squash over a
    ns = np.sum(pre * pre, axis=1, keepdims=True)
    return pre * np.sqrt(ns) / (1.0 + ns)


def _device_softmax_route(nc, pools, logits_g, g):
    """softmax over o (innermost 64) of logits_g (128,64) fp32 -> route (128,64) f16."""
    expp, zsum, rcp, rpool = pools
    e = expp.tile([128, O], F16)
    nc.scalar.activation(e[:, :], logits_g, ACTFN.Exp)
    z = zsum.tile([128, 1], F32)
    nc.vector.tensor_reduce(z[:, :], e[:, :], axis=AX.X, op=ALU.add)
    zr = rcp.tile([128, 1], F32)
    nc.vector.reciprocal(zr[:, :], z[:, :])
    r = rpool.tile([128, O], F16)
    # route = exp * (1/Z)  on ACT (per-partition scalar scale)
    nc.scalar.activation(r[:, :], e[:, :], ACTFN.Copy, scale=zr[:, :])
    return r


def build_nc():
    nc = bacc.Bacc("TRN2", target_bir_lowering=False, debug=False, num_devices=NCORES)

    w_d = nc.declare_dram_parameter("w", [G // 2, 128, 2, J], F16, isOutput=False)
    xbd_d = nc.declare_dram_parameter("xbd", [128, G, 128], F16, isOutput=False)
    xdn_d = nc.declare_dram_parameter("xdn", [128, G, B], F16, isOutput=False)
    ones_d = nc.declare_dram_parameter("onesbd", [128, B], F16, isOutput=False)
    dup_d = nc.declare_dram_parameter("dup16", [B, 128], F16, isOutput=False)
    brow_d = nc.declare_dram_parameter("biasrow", [1, J], F16, isOutput=False)
    blhs_d = nc.declare_dram_parameter("biaslhs", [1, B], F16, isOutput=False)
    out_d = nc.declare_dram_parameter("partial", [B, J], F32, isOutput=True)

    # collective bounce buffers (internal DRAM; outputs in Shared space)
    cc_in = [nc.dram_tensor(f"cc_in{t}", [B, J], F32) for t in range(2)]
    cc_out = [
        nc.dram_tensor(f"cc_out{t}", [B, J], F32, addr_space="Shared") for t in range(2)
    ]
    rg = [list(range(NCORES))]

    with tile.TileContext(nc) as tc:
        with (
            tc.tile_pool(name="const", bufs=1) as constp,
            tc.tile_pool(name="l1", bufs=1) as l1p,
            tc.tile_pool(name="wst", bufs=2) as wp,
            tc.tile_pool(name="mmps", bufs=4, space="PSUM") as mmps,
            tc.tile_pool(name="preps", bufs=1, space="PSUM") as preps,
            tc.tile_pool(name="dtmp", bufs=2) as dpool,
            tc.tile_pool(name="s1", bufs=1) as s1p,
            tc.tile_pool(name="s2", bufs=1) as s2p,
            tc.tile_pool(name="s3", bufs=1) as s3p,
            tc.tile_pool(name="s4", bufs=1) as s4p,
            tc.tile_pool(name="expp", bufs=2) as expp,
            tc.tile_pool(name="zsum", bufs=2) as zsum,
            tc.tile_pool(name="rcp", bufs=2) as rcp,
            tc.tile_pool(name="route", bufs=2) as routep,
            tc.tile_pool(name="small", bufs=1) as smallp,
            tc.tile_pool(name="actbx", bufs=1) as actbxp,
            tc.tile_pool(name="logits", bufs=1) as logitsp,
        ):
            softmax_pools = (expp, zsum, rcp, routep)

            # ---- constants ----
            xbd = constp.tile([128, G, 128], F16)
            nc.sync.dma_start(xbd[:, :, :], xbd_d[:, :, :])
            xdn = constp.tile([128, G, B], F16)
            nc.sync.dma_start(xdn[:, :, :], xdn_d[:, :, :])
            onesbd = constp.tile([128, B], F16)
            nc.sync.dma_start(onesbd[:, :], ones_d[:, :])
            dup16 = constp.tile([B, 128], F16)
            nc.sync.dma_start(dup16[:, :], dup_d[:, :])
            biasrow = constp.tile([1, J], F16)
            nc.sync.dma_start(biasrow[:, :], brow_d[:, :])
            biaslhs = constp.tile([1, B], F16)
            nc.sync.dma_start(biaslhs[:, :], blhs_d[:, :])

            L1 = l1p.tile([128, G, J], F16)  # resident votes, 16 MB
            logits = logitsp.tile([128, G, O], F16)

            # ================= P0a: pre1 partial only (W stream 1) ==========
            pre_ps = preps.tile([B, J], F32, tag="pre")
            for gp in range(G // 2):
                wt = wp.tile([128, 2, J], F16, tag="wt")
                nc.sync.dma_start(wt[:, :, :], w_d[gp, :, :, :])
                for gi in range(2):
                    g = 2 * gp + gi
                    for c in range(4):
                        cs = slice(c * 512, (c + 1) * 512)
                        # pre1 partial: uniform-route sum (xdn pre-scaled 1/64)
                        nc.tensor.matmul(
                            pre_ps[:, cs],
                            xdn[:, g, :],
                            wt[:, gi, cs],
                            start=(g == 0),
                            stop=False,
                        )
            # fold bias/NCORES into the partial so squash skips the bias add
            for c in range(4):
                cs = slice(c * 512, (c + 1) * 512)
                nc.tensor.matmul(
                    pre_ps[:, cs],
                    biaslhs[:, :],
                    biasrow[:, cs],
                    start=False,
                    stop=True,
                )

            # ================= P0b: votes production (W stream 2) ===========
            def produce_votes():
                for gp in range(G // 2):
                    wt = wp.tile([128, 2, J], F16, tag="wt")
                    nc.sync.dma_start(wt[:, :, :], w_d[gp, :, :, :])
                    for gi in range(2):
                        g = 2 * gp + gi
                        for c in range(4):
                            cs = slice(c * 512, (c + 1) * 512)
                            pm = mmps.tile([128, 512], F32, tag="pm")
                            nc.tensor.matmul(
                                pm[:, :], xbd[:, g, :], wt[:, gi, cs],
                                start=True, stop=True,
                            )
                            if c % 2 == 0:
                                nc.vector.tensor_copy(L1[:, g, cs], pm[:, :])
                            else:
                                nc.scalar.copy(L1[:, g, cs], pm[:, :])

            # ================= iteration boundaries =================
            actbx = actbxp.tile([128, J], F16)

            def squash_to_actbx(cc_out_t):
                """DMA AR result in, + bias, squash, then broadcast to 128 partitions."""
                pre_sb = smallp.tile([B, J], F32, tag="pre_sb")
                nc.sync.dma_start(pre_sb[:, :], cc_out_t[:, :])
                sq = smallp.tile([B, J], F32, tag="preout")
                nc.scalar.activation(sq[:, :], pre_sb[:, :], ACTFN.Square)
                ns = smallp.tile([B, O], F32, tag="ns")
                nc.vector.tensor_reduce(
                    ns[:, :],
                    sq[:, :].rearrange("p (a o) -> p o a", a=A),
                    axis=AX.X,
                    op=ALU.add,
                )
                rt = smallp.tile([B, O], F32, tag="rt")
                nc.scalar.activation(rt[:, :], ns[:, :], ACTFN.Sqrt)
                # one Newton step: rt2 = 0.5*(rt + ns/rt)
                rci = smallp.tile([B, O], F32, tag="rci")
                nc.vector.reciprocal(rci[:, :], rt[:, :])
                nc.vector.tensor_mul(rci[:, :], rci[:, :], ns[:, :])
                nc.vector.tensor_add(rci[:, :], rci[:, :], rt[:, :])
                den = smallp.tile([B, O], F32, tag="den")
                nc.vector.tensor_scalar_add(den[:, :], ns[:, :], 1.0)
                nc.vector.reciprocal(den[:, :], den[:, :])
                s = smallp.tile([B, O], F32, tag="s")
                nc.vector.tensor_mul(s[:, :], den[:, :], rci[:, :])
                nc.vector.tensor_scalar_mul(s[:, :], s[:, :], 0.5)
                act16 = smallp.tile([B, J], F16, tag="act16")
                nc.vector.tensor_mul(
                    act16[:, :].rearrange("p (a o) -> p a o", a=A),
                    pre_sb[:, :].rearrange("p (a o) -> p a o", a=A),
                    s[:, :].rearrange("p (u o) -> p u o", u=1).broadcast_to((B, A, O)),
                )
                # broadcast act to (b,i)-partition layout via dup matmul
                for c in range(4):
                    cs = slice(c * 512, (c + 1) * 512)
                    pm = mmps.tile([128, 512], F32)
                    nc.tensor.matmul(
                        pm[:, :], dup16[:, :], act16[:, cs], start=True, stop=True
                    )
                    if c % 2 == 0:
                        nc.vector.tensor_copy(actbx[:, cs], pm[:, :])
                    else:
                        nc.scalar.copy(actbx[:, cs], pm[:, :])

            def iteration(t, pre_ps_prev, first_dist):
                """Evac prev pre partial -> AR -> squash -> distances+route+next pre."""
                pre_sb_out = smallp.tile([B, J], F32, tag="preout")
                nc.scalar.copy(pre_sb_out[:, :], pre_ps_prev[:, :])
                nc.sync.dma_start(cc_in[t][:, :], pre_sb_out[:, :])
                nc.gpsimd.collective_compute(
                    "AllReduce",
                    ALU.add,
                    replica_groups=rg,
                    ins=[cc_in[t][:, :]],
                    outs=[cc_out[t][:, :]],
                )
                if t == 0:
                    produce_votes()  # overlaps AR1: no dependency on act1
                squash_to_actbx(cc_out[t])

                pre_ps_next = preps.tile([B, J], F32, tag="pre")
                for g in range(G):
                    dt = dpool.tile([128, J], F16, tag="dtmp")
                    nc.vector.tensor_mul(dt[:, :], L1[:, g, :], actbx[:, :])
                    s1 = s1p.tile([128, 1024], F16)
                    nc.vector.tensor_add(s1[:, :], dt[:, :1024], dt[:, 1024:])
                    s2 = s2p.tile([128, 512], F16)
                    nc.vector.tensor_add(s2[:, :], s1[:, :512], s1[:, 512:])
                    s3 = s3p.tile([128, 256], F16)
                    nc.vector.tensor_add(s3[:, :], s2[:, :256], s2[:, 256:])
                    s4 = s4p.tile([128, 128], F16)
                    nc.vector.tensor_add(s4[:, :], s3[:, :128], s3[:, 128:])
                    if first_dist:
                        nc.vector.tensor_add(
                            logits[:, g, :], s4[:, :64], s4[:, 64:]
                        )
                    else:
                        s5 = s4p.tile([128, 64], F16, tag="s5")
                        nc.vector.tensor_add(s5[:, :], s4[:, :64], s4[:, 64:])
                        nc.vector.tensor_add(logits[:, g, :], logits[:, g, :], s5[:, :])
                    r = _device_softmax_route(nc, softmax_pools, logits[:, g, :], g)
                    wv = dpool.tile([128, J], F16, tag="wv")
                    nc.vector.tensor_mul(
                        wv[:, :].rearrange("p (a o) -> p a o", a=A),
                        L1[:, g, :].rearrange("p (a o) -> p a o", a=A),
                        r[:, :].rearrange("p (u o) -> p u o", u=1).broadcast_to(
                            (128, A, O)
                        ),
                    )
                    for c in range(4):
                        cs = slice(c * 512, (c + 1) * 512)
                        nc.tensor.matmul(
                            pre_ps_next[:, cs],
                            onesbd[:, :],
                            wv[:, cs],
                            start=(g == 0),
                            stop=False,
                        )
                for c in range(4):
                    cs = slice(c * 512, (c + 1) * 512)
                    nc.tensor.matmul(
                        pre_ps_next[:, cs],
                        biaslhs[:, :],
                        biasrow[:, cs],
                        start=False,
                        stop=True,
                    )
                return pre_ps_next

            pre2_ps = iteration(0, pre_ps, first_dist=True)
            pre3_ps = iteration(1, pre2_ps, first_dist=False)

            out_sb = smallp.tile([B, J], F32, tag="preout")
            nc.scalar.copy(out_sb[:, :], pre3_ps[:, :])
            nc.sync.dma_start(out_d[:, :], out_sb[:, :])

    nc.finalize()
    return nc


_NC_CACHE = None


def _get_nc():
    global _NC_CACHE
    if _NC_CACHE is None:
        _NC_CACHE = build_nc()
    return _NC_CACHE


def prepare_inputs(x, weights):
    """Host-side sharding and layout prep. Returns list of per-core input dicts."""
    x = np.asarray(x, np.float32)[..., 0]  # (B, I, K)
    W = np.asarray(weights, np.float32)  # (I, K, J) with j = o*A + a

    # j' = a*64 + o  (a outer, o inner)
    Wp = (
        W.reshape(I, K, O, A).transpose(0, 1, 3, 2).reshape(I, K, J).astype(np.float16)
    )

    onesbd = np.zeros((128, B), np.float16)
    dup16 = np.zeros((B, 128), np.float16)
    for b in range(B):
        onesbd[b * 8 : (b + 1) * 8, b] = 1.0
        dup16[b, b * 8 : (b + 1) * 8] = 1.0

    in_maps = []
    for c in range(NCORES):
        xs = x[:, c * ILOC : (c + 1) * ILOC, :]  # (B, 256, K)
        # w: (G, 128, J) with row p = isub*16 + k
        wc = Wp[c * ILOC : (c + 1) * ILOC].reshape(G, 8 * K, J)
        wc = wc.reshape(G // 2, 2, 128, J).transpose(0, 2, 1, 3)
        # xbd: (128, G, 128): [isub*16+k, g, b*8+isub'] = x[b, 8g+isub, k] iff isub==isub'
        xbd = np.zeros((128, G, 128), np.float16)
        xdn = np.zeros((128, G, B), np.float16)
        xg = xs.reshape(B, G, 8, K)  # b, g, isub, k
        for isub in range(8):
            # rows isub*16 : isub*16+16, cols b*8+isub
            xbd[isub * K : (isub + 1) * K, :, isub::8] = xg[:, :, isub, :].transpose(
                2, 1, 0
            )
            xdn[isub * K : (isub + 1) * K, :, :] = (
                xg[:, :, isub, :].transpose(2, 1, 0) / 64.0
            )
        in_maps.append(
            {
                "w": np.ascontiguousarray(wc),
                "xbd": xbd,
                "xdn": xdn,
                "onesbd": onesbd,
                "dup16": dup16,
                "biasrow": np.zeros((1, J), np.float16),  # placeholder
                "biaslhs": np.full((1, B), 1.0 / NCORES, np.float16),
            }
        )
    return in_maps


def kernel(x, weights, bias):
    bias = np.asarray(bias, np.float32)  # (O, A)
    in_maps = prepare_inputs(x, weights)
    biasb = np.broadcast_to(
        bias.T.reshape(1, J), (B, J)
    ).copy()  # j' = a*64+o -> bias.T is (A, O)
    for m in in_maps:
        m["biasrow"] = biasb[:1].astype(np.float16)

    nc = _get_nc()
    res = run_bass_kernel_spmd(nc, in_maps, core_ids=list(range(NCORES)))
    partials = [res.results[c]["partial"] for c in range(NCORES)]

    total = np.sum(np.stack(partials, 0), axis=0, dtype=np.float64).astype(np.float32)
    pre3 = total.reshape(B, A, O)
    act = _squash_host(pre3)  # (B, A, O)
    return np.ascontiguousarray(act.transpose(0, 2, 1))  # (B, O, A)


# revision 29
# speedup vs baseline: 31.1020x; 1.0172x over previous
"""Trainium2 Bass kernel for nn_DigitCap (CapsNet DigitCaps dynamic routing).

Computation (forward only, stop_gradient is a no-op for values):
    votes[b,i,o,a] = sum_k x[b,i,k] * W[i,k,(o,a)]          # B=16, I=2048, K=16, O=64, A=32
    logits = 0
    for it in 1..3:
        route = softmax_o(logits)
        pre[b,o,a] = sum_i route[b,i,o]*votes[b,i,o,a] + bias
        act = squash_a(pre)
        if it < 3: logits += sum_a votes[b,i,o,a]*act[b,o,a]
    return act

Distribution: shard I across 8 cores (256 capsules each).  Weights are read
once per core (16 MB fp16 slice), votes stay resident in SBUF in fp16.
The only cross-core coupling is the i-sum inside `pre`: two in-kernel
AllReduces of the 128 KB partial (iterations 1 and 2).  The final
iteration's partial is returned per-core and reduced + squashed on host.

On-device layout: j' = a*64 + o (a-outer) so that
  - softmax / squash reductions are clean free-dim group reductions
  - the distances a-reduction is a contiguous-halves TT-add tree
Partition layout of votes: p = b*8 + i_sub (b-outer) over groups g of 8
capsules; produced by a block-diagonal stationary x so each moving W column
feeds 128 useful MACs.
"""

import sys

sys.path.insert(0, "/opt/trn_rl_repo")

import numpy as np

import concourse.bass as bass
import concourse.bacc as bacc
import concourse.mybir as mybir
from concourse import tile
from concourse.bass_utils import run_bass_kernel_spmd

B = 16
I = 2048
K = 16  # input atoms
O = 64
A = 32  # output atoms
J = 2048  # O*A
NCORES = 8
ILOC = I // NCORES  # 256
G = ILOC // 8  # 32 groups of 8 capsules

F16 = mybir.dt.float16
F32 = mybir.dt.float32
AX = mybir.AxisListType
ALU = mybir.AluOpType
ACTFN = mybir.ActivationFunctionType


def _squash_host(pre):
    # pre: (B, A, O) in j' order (a outer, o inner); squash over a
    ns = np.sum(pre * pre, axis=1, keepdims=True)
    return pre * np.sqrt(ns) / (1.0 + ns)


def _device_softmax_route(nc, pools, logits_g, g):
    """softmax over o (innermost 64) of logits_g (128,64) fp32 -> route (128,64) f16."""
    expp, zsum, rcp, rpool = pools
    e = expp.tile([128, O], F16)
    nc.scalar.activation(e[:, :], logits_g, ACTFN.Exp)
    z = zsum.tile([128, 1], F32)
    nc.vector.tensor_reduce(z[:, :], e[:, :], axis=AX.X, op=ALU.add)
    zr = rcp.tile([128, 1], F32)
    nc.vector.reciprocal(zr[:, :], z[:, :])
    r = rpool.tile([128, O], F16)
    # route = exp * (1/Z)  on ACT (per-partition scalar scale)
    nc.scalar.activation(r[:, :], e[:, :], ACTFN.Copy, scale=zr[:, :])
    return r


def build_nc():
    nc = bacc.Bacc("TRN2", target_bir_lowering=False, debug=False, num_devices=NCORES)

    w_d = nc.declare_dram_parameter("w", [G // 4, 128, 4, J], F16, isOutput=False)
    xbd_d = nc.declare_dram_parameter("xbd", [128, G, 128], F16, isOutput=False)
    xdn_d = nc.declare_dram_parameter("xdn", [128, G, B], F16, isOutput=False)
    ones_d = nc.declare_dram_parameter("onesbd", [128, B], F16, isOutput=False)
    dup_d = nc.declare_dram_parameter("dup16", [B, 128], F16, isOutput=False)
    brow_d = nc.declare_dram_parameter("biasrow", [1, J], F16, isOutput=False)
    blhs_d = nc.declare_dram_parameter("biaslhs", [1, B], F16, isOutput=False)
    out_d = nc.declare_dram_parameter("partial", [B, J], F32, isOutput=True)

    # collective bounce buffers (internal DRAM; outputs in Shared space)
    cc_in = [nc.dram_tensor(f"cc_in{t}", [B, J], F32) for t in range(2)]
    cc_out = [
        nc.dram_tensor(f"cc_out{t}", [B, J], F32, addr_space="Shared") for t in range(2)
    ]
    rg = [list(range(NCORES))]

    with tile.TileContext(nc) as tc:
        with (
            tc.tile_pool(name="const", bufs=1) as constp,
            tc.tile_pool(name="l1", bufs=1) as l1p,
            tc.tile_pool(name="mmps", bufs=4, space="PSUM") as mmps,
            tc.tile_pool(name="preps", bufs=1, space="PSUM") as preps,
            tc.tile_pool(name="expp", bufs=2) as expp,
            tc.tile_pool(name="zsum", bufs=2) as zsum,
            tc.tile_pool(name="rcp", bufs=2) as rcp,
            tc.tile_pool(name="route", bufs=2) as routep,
            tc.tile_pool(name="small", bufs=1) as smallp,
            tc.tile_pool(name="actbx", bufs=1) as actbxp,
            tc.tile_pool(name="logits", bufs=1) as logitsp,
        ):
            softmax_pools = (expp, zsum, rcp, routep)

            # ---- constants ----
            xbd = constp.tile([128, G, 128], F16)
            nc.sync.dma_start(xbd[:, :, :], xbd_d[:, :, :])
            xdn = constp.tile([128, G, B], F16)
            nc.sync.dma_start(xdn[:, :, :], xdn_d[:, :, :])
            onesbd = constp.tile([128, B], F16)
            nc.sync.dma_start(onesbd[:, :], ones_d[:, :])
            dup16 = constp.tile([B, 128], F16)
            nc.sync.dma_start(dup16[:, :], dup_d[:, :])
            biasrow = constp.tile([1, J], F16)
            nc.sync.dma_start(biasrow[:, :], brow_d[:, :])
            biaslhs = constp.tile([1, B], F16)
            nc.sync.dma_start(biaslhs[:, :], blhs_d[:, :])

            L1 = l1p.tile([128, G, J], F16)  # resident votes, 16 MB
            logits = logitsp.tile([128, G, O], F16)

            # ================= P0a: pre1 partial only (W stream 1) ==========
            wscope = tc.tile_pool(name="wst", bufs=2)
            wp = wscope.__enter__()
            pre_ps = preps.tile([B, J], F32, tag="pre")
            for gp in range(G // 4):
                wt = wp.tile([128, 4, J], F16, tag="wt")
                nc.sync.dma_start(wt[:, :, :], w_d[gp, :, :, :])
                for gi in range(4):
                    g = 4 * gp + gi
                    for c in range(4):
                        cs = slice(c * 512, (c + 1) * 512)
                        # pre1 partial: uniform-route sum (xdn pre-scaled 1/64)
                        nc.tensor.matmul(
                            pre_ps[:, cs],
                            xdn[:, g, :],
                            wt[:, gi, cs],
                            start=(g == 0),
                            stop=False,
                        )
            # fold bias/NCORES into the partial so squash skips the bias add
            for c in range(4):
                cs = slice(c * 512, (c + 1) * 512)
                nc.tensor.matmul(
                    pre_ps[:, cs],
                    biaslhs[:, :],
                    biasrow[:, cs],
                    start=False,
                    stop=True,
                )

            # ================= P0b: votes production (W stream 2) ===========
            def produce_votes():
                for gp in range(G // 4):
                    wt = wp.tile([128, 4, J], F16, tag="wt")
                    nc.sync.dma_start(wt[:, :, :], w_d[gp, :, :, :])
                    for gi in range(4):
                        g = 4 * gp + gi
                        for c in range(4):
                            cs = slice(c * 512, (c + 1) * 512)
                            pm = mmps.tile([128, 512], F32, tag="pm")
                            nc.tensor.matmul(
                                pm[:, :], xbd[:, g, :], wt[:, gi, cs],
                                start=True, stop=True,
                            )
                            if c % 2 == 0:
                                nc.vector.tensor_copy(L1[:, g, cs], pm[:, :])
                            else:
                                nc.scalar.copy(L1[:, g, cs], pm[:, :])

            # ================= iteration boundaries =================
            actbx = actbxp.tile([128, J], F16)

            def squash_to_actbx(cc_out_t):
                """DMA AR result in, + bias, squash, then broadcast to 128 partitions."""
                pre_sb = smallp.tile([B, J], F32, tag="pre_sb")
                nc.sync.dma_start(pre_sb[:, :], cc_out_t[:, :])
                sq = smallp.tile([B, J], F32, tag="preout")
                nc.scalar.activation(sq[:, :], pre_sb[:, :], ACTFN.Square)
                ns = smallp.tile([B, O], F32, tag="ns")
                nc.vector.tensor_reduce(
                    ns[:, :],
                    sq[:, :].rearrange("p (a o) -> p o a", a=A),
                    axis=AX.X,
                    op=ALU.add,
                )
                # sqrt(ns) = exp(0.5*ln(ns)): stays in the natural_log_exp
                # ACT table set that softmax Exp uses (no ~2.7us set reloads),
                # and is more accurate than the Sqrt spline (65536-ULP budget).
                rt = smallp.tile([B, O], F32, tag="rt")
                nc.scalar.activation(rt[:, :], ns[:, :], ACTFN.Ln)
                rci = smallp.tile([B, O], F32, tag="rci")
                nc.scalar.activation(rci[:, :], rt[:, :], ACTFN.Exp, scale=0.5)
                den = smallp.tile([B, O], F32, tag="den")
                nc.vector.tensor_scalar_add(den[:, :], ns[:, :], 1.0)
                nc.vector.reciprocal(den[:, :], den[:, :])
                s = smallp.tile([B, O], F32, tag="s")
                nc.vector.tensor_mul(s[:, :], den[:, :], rci[:, :])
                act16 = smallp.tile([B, J], F16, tag="act16")
                nc.vector.tensor_mul(
                    act16[:, :].rearrange("p (a o) -> p a o", a=A),
                    pre_sb[:, :].rearrange("p (a o) -> p a o", a=A),
                    s[:, :].rearrange("p (u o) -> p u o", u=1).broadcast_to((B, A, O)),
                )
                # broadcast act to (b,i)-partition layout via dup matmul
                for c in range(4):
                    cs = slice(c * 512, (c + 1) * 512)
                    pm = mmps.tile([128, 512], F32)
                    nc.tensor.matmul(
                        pm[:, :], dup16[:, :], act16[:, cs], start=True, stop=True
                    )
                    if c % 2 == 0:
                        nc.vector.tensor_copy(actbx[:, cs], pm[:, :])
                    else:
                        nc.scalar.copy(actbx[:, cs], pm[:, :])

            def start_allreduce(t, pre_ps_prev):
                pre_sb_out = smallp.tile([B, J], F32, tag="preout")
                nc.scalar.copy(pre_sb_out[:, :], pre_ps_prev[:, :])
                nc.sync.dma_start(cc_in[t][:, :], pre_sb_out[:, :])
                nc.gpsimd.collective_compute(
                    "AllReduce",
                    ALU.add,
                    replica_groups=rg,
                    ins=[cc_in[t][:, :]],
                    outs=[cc_out[t][:, :]],
                )

            # AR1 overlaps the votes production (no dependency on act1);
            # the W streaming pool closes before iteration scratch pools open.
            start_allreduce(0, pre_ps)
            produce_votes()
            wscope.__exit__(None, None, None)

            itstack = [
                tc.tile_pool(name="dtmp", bufs=2),
                tc.tile_pool(name="s1", bufs=1),
                tc.tile_pool(name="s2", bufs=1),
                tc.tile_pool(name="s3", bufs=1),
                tc.tile_pool(name="s4", bufs=1),
            ]
            dpool, s1p, s2p, s3p, s4p = [p.__enter__() for p in itstack]

            def iteration(t, first_dist):
                """squash(AR result) -> distances+route+next pre partial."""
                squash_to_actbx(cc_out[t])

                pre_ps_next = preps.tile([B, J], F32, tag="pre")

                def dist_part(g):
                    dt = dpool.tile([128, J], F16, tag="dtmp")
                    nc.vector.tensor_mul(dt[:, :], L1[:, g, :], actbx[:, :])
                    s1 = s1p.tile([128, 1024], F16)
                    nc.vector.tensor_add(s1[:, :], dt[:, :1024], dt[:, 1024:])
                    s2 = s2p.tile([128, 512], F16)
                    nc.vector.tensor_add(s2[:, :], s1[:, :512], s1[:, 512:])
                    s3 = s3p.tile([128, 256], F16)
                    nc.vector.tensor_add(s3[:, :], s2[:, :256], s2[:, 256:])
                    s4 = s4p.tile([128, 128], F16)
                    nc.vector.tensor_add(s4[:, :], s3[:, :128], s3[:, 128:])
                    if first_dist:
                        nc.vector.tensor_add(
                            logits[:, g, :], s4[:, :64], s4[:, 64:]
                        )
                    else:
                        s5 = s4p.tile([128, 64], F16, tag="s5")
                        nc.vector.tensor_add(s5[:, :], s4[:, :64], s4[:, 64:])
                        nc.vector.tensor_add(logits[:, g, :], logits[:, g, :], s5[:, :])

                def route_part(g):
                    r = _device_softmax_route(nc, softmax_pools, logits[:, g, :], g)
                    wv = dpool.tile([128, J], F16, tag="wv")
                    nc.vector.tensor_mul(
                        wv[:, :].rearrange("p (a o) -> p a o", a=A),
                        L1[:, g, :].rearrange("p (a o) -> p a o", a=A),
                        r[:, :].rearrange("p (u o) -> p u o", u=1).broadcast_to(
                            (128, A, O)
                        ),
                    )
                    for c in range(4):
                        cs = slice(c * 512, (c + 1) * 512)
                        nc.tensor.matmul(
                            pre_ps_next[:, cs],
                            onesbd[:, :],
                            wv[:, cs],
                            start=(g == 0),
                            stop=False,
                        )

                # 1-group software pipeline: softmax/wv of g-1 issues while
                # DVE streams g's distance chain, hiding the ACT round trips.
                for g in range(G):
                    dist_part(g)
                    if g >= 1:
                        route_part(g - 1)
                route_part(G - 1)
                for c in range(4):
                    cs = slice(c * 512, (c + 1) * 512)
                    nc.tensor.matmul(
                        pre_ps_next[:, cs],
                        biaslhs[:, :],
                        biasrow[:, cs],
                        start=False,
                        stop=True,
                    )
                return pre_ps_next

            pre2_ps = iteration(0, first_dist=True)
            start_allreduce(1, pre2_ps)
            pre3_ps = iteration(1, first_dist=False)

            out_sb = smallp.tile([B, J], F32, tag="preout")
            nc.scalar.copy(out_sb[:, :], pre3_ps[:, :])
            nc.sync.dma_start(out_d[:, :], out_sb[:, :])
            for p in reversed(itstack):
                p.__exit__(None, None, None)

    nc.finalize()
    return nc


_NC_CACHE = None


def _get_nc():
    global _NC_CACHE
    if _NC_CACHE is None:
        _NC_CACHE = build_nc()
    return _NC_CACHE


def prepare_inputs(x, weights):
    """Host-side sharding and layout prep. Returns list of per-core input dicts."""
    x = np.asarray(x, np.float32)[..., 0]  # (B, I, K)
    W = np.asarray(weights, np.float32)  # (I, K, J) with j = o*A + a

    # j' = a*64 + o  (a outer, o inner)
    Wp = (
        W.reshape(I, K, O, A).transpose(0, 1, 3, 2).reshape(I, K, J).astype(np.float16)
    )

    onesbd = np.zeros((128, B), np.float16)
    dup16 = np.zeros((B, 128), np.float16)
    for b in range(B):
        onesbd[b * 8 : (b + 1) * 8, b] = 1.0
        dup16[b, b * 8 : (b + 1) * 8] = 1.0

    in_maps = []
    for c in range(NCORES):
        xs = x[:, c * ILOC : (c + 1) * ILOC, :]  # (B, 256, K)
        # w: (G, 128, J) with row p = isub*16 + k
        wc = Wp[c * ILOC : (c + 1) * ILOC].reshape(G, 8 * K, J)
        wc = wc.reshape(G // 4, 4, 128, J).transpose(0, 2, 1, 3)
        # xbd: (128, G, 128): [isub*16+k, g, b*8+isub'] = x[b, 8g+isub, k] iff isub==isub'
        xbd = np.zeros((128, G, 128), np.float16)
        xdn = np.zeros((128, G, B), np.float16)
        xg = xs.reshape(B, G, 8, K)  # b, g, isub, k
        for isub in range(8):
            # rows isub*16 : isub*16+16, cols b*8+isub
            xbd[isub * K : (isub + 1) * K, :, isub::8] = xg[:, :, isub, :].transpose(
                2, 1, 0
            )
            xdn[isub * K : (isub + 1) * K, :, :] = (
                xg[:, :, isub, :].transpose(2, 1, 0) / 64.0
            )
        in_maps.append(
            {
                "w": np.ascontiguousarray(wc),
                "xbd": xbd,
                "xdn": xdn,
                "onesbd": onesbd,
                "dup16": dup16,
                "biasrow": np.zeros((1, J), np.float16),  # placeholder
                "biaslhs": np.full((1, B), 1.0 / NCORES, np.float16),
            }
        )
    return in_maps


def kernel(x, weights, bias):
    bias = np.asarray(bias, np.float32)  # (O, A)
    in_maps = prepare_inputs(x, weights)
    biasb = np.broadcast_to(
        bias.T.reshape(1, J), (B, J)
    ).copy()  # j' = a*64+o -> bias.T is (A, O)
    for m in in_maps:
        m["biasrow"] = biasb[:1].astype(np.float16)

    nc = _get_nc()
    res = run_bass_kernel_spmd(nc, in_maps, core_ids=list(range(NCORES)))
    partials = [res.results[c]["partial"] for c in range(NCORES)]

    total = np.sum(np.stack(partials, 0), axis=0, dtype=np.float64).astype(np.float32)
    pre3 = total.reshape(B, A, O)
    act = _squash_host(pre3)  # (B, A, O)
    return np.ascontiguousarray(act.transpose(0, 2, 1))  # (B, O, A)


# revision 32
# speedup vs baseline: 31.2300x; 1.0041x over previous
"""Trainium2 Bass kernel for nn_DigitCap (CapsNet DigitCaps dynamic routing).

Computation (forward only, stop_gradient is a no-op for values):
    votes[b,i,o,a] = sum_k x[b,i,k] * W[i,k,(o,a)]          # B=16, I=2048, K=16, O=64, A=32
    logits = 0
    for it in 1..3:
        route = softmax_o(logits)
        pre[b,o,a] = sum_i route[b,i,o]*votes[b,i,o,a] + bias
        act = squash_a(pre)
        if it < 3: logits += sum_a votes[b,i,o,a]*act[b,o,a]
    return act

Distribution: shard I across 8 cores (256 capsules each).  Weights are read
once per core (16 MB fp16 slice), votes stay resident in SBUF in fp16.
The only cross-core coupling is the i-sum inside `pre`: two in-kernel
AllReduces of the 128 KB partial (iterations 1 and 2).  The final
iteration's partial is returned per-core and reduced + squashed on host.

On-device layout: j' = a*64 + o (a-outer) so that
  - softmax / squash reductions are clean free-dim group reductions
  - the distances a-reduction is a contiguous-halves TT-add tree
Partition layout of votes: p = b*8 + i_sub (b-outer) over groups g of 8
capsules; produced by a block-diagonal stationary x so each moving W column
feeds 128 useful MACs.
"""

import sys

sys.path.insert(0, "/opt/trn_rl_repo")

import numpy as np

import concourse.bass as bass
import concourse.bacc as bacc
import concourse.mybir as mybir
from concourse import tile
from concourse.bass_utils import run_bass_kernel_spmd

B = 16
I = 2048
K = 16  # input atoms
O = 64
A = 32  # output atoms
J = 2048  # O*A
NCORES = 8
ILOC = I // NCORES  # 256
G = ILOC // 8  # 32 groups of 8 capsules

F16 = mybir.dt.float16
F32 = mybir.dt.float32
AX = mybir.AxisListType
ALU = mybir.AluOpType
ACTFN = mybir.ActivationFunctionType


def _squash_host(pre):
    # pre: (B, A, O) in j' order (a outer, o inner); squash over a
    ns = np.sum(pre * pre, axis=1, keepdims=True)
    return pre * np.sqrt(ns) / (1.0 + ns)


def _device_softmax_route(nc, pools, logits_g, g):
    """softmax over o (innermost 64) of logits_g (128,64) fp32 -> route (128,64) f16."""
    expp, zsum, rcp, rpool = pools
    e = expp.tile([128, O], F16)
    nc.scalar.activation(e[:, :], logits_g, ACTFN.Exp)
    z = zsum.tile([128, 1], F32)
    nc.vector.tensor_reduce(z[:, :], e[:, :], axis=AX.X, op=ALU.add)
    zr = rcp.tile([128, 1], F32)
    nc.vector.reciprocal(zr[:, :], z[:, :])
    r = rpool.tile([128, O], F16)
    # route = exp * (1/Z)  on ACT (per-partition scalar scale)
    nc.scalar.activation(r[:, :], e[:, :], ACTFN.Copy, scale=zr[:, :])
    return r


def build_nc():
    nc = bacc.Bacc("TRN2", target_bir_lowering=False, debug=False, num_devices=NCORES)

    w_d = nc.declare_dram_parameter("w", [G // 4, 128, 4, J], F16, isOutput=False)
    xbd_d = nc.declare_dram_parameter("xbd", [128, G, 128], F16, isOutput=False)
    xdn_d = nc.declare_dram_parameter("xdn", [128, G, B], F16, isOutput=False)
    ones_d = nc.declare_dram_parameter("onesbd", [128, B], F16, isOutput=False)
    dup_d = nc.declare_dram_parameter("dup16", [B, 128], F16, isOutput=False)
    brow_d = nc.declare_dram_parameter("biasrow", [1, J], F16, isOutput=False)
    blhs_d = nc.declare_dram_parameter("biaslhs", [1, B], F16, isOutput=False)
    out_d = nc.declare_dram_parameter("partial", [B, J], F32, isOutput=True)

    # collective bounce buffers (internal DRAM; outputs in Shared space)
    cc_in = [nc.dram_tensor(f"cc_in{t}", [B, J], F32) for t in range(2)]
    cc_out = [
        nc.dram_tensor(f"cc_out{t}", [B, J], F32, addr_space="Shared") for t in range(2)
    ]
    rg = [list(range(NCORES))]

    with tile.TileContext(nc) as tc:
        with (
            tc.tile_pool(name="const", bufs=1) as constp,
            tc.tile_pool(name="l1", bufs=1) as l1p,
            tc.tile_pool(name="mmps", bufs=4, space="PSUM") as mmps,
            tc.tile_pool(name="preps", bufs=1, space="PSUM") as preps,
            tc.tile_pool(name="expp", bufs=2) as expp,
            tc.tile_pool(name="zsum", bufs=2) as zsum,
            tc.tile_pool(name="rcp", bufs=2) as rcp,
            tc.tile_pool(name="route", bufs=2) as routep,
            tc.tile_pool(name="small", bufs=1) as smallp,
            tc.tile_pool(name="actbx", bufs=1) as actbxp,
            tc.tile_pool(name="logits", bufs=1) as logitsp,
        ):
            softmax_pools = (expp, zsum, rcp, routep)

            # ---- constants ----
            xbd = constp.tile([128, G, 128], F16)
            nc.sync.dma_start(xbd[:, :, :], xbd_d[:, :, :])
            xdn = constp.tile([128, G, B], F16)
            nc.sync.dma_start(xdn[:, :, :], xdn_d[:, :, :])
            onesbd = constp.tile([128, B], F16)
            nc.sync.dma_start(onesbd[:, :], ones_d[:, :])
            dup16 = constp.tile([B, 128], F16)
            nc.sync.dma_start(dup16[:, :], dup_d[:, :])
            biasrow = constp.tile([1, J], F16)
            nc.sync.dma_start(biasrow[:, :], brow_d[:, :])
            biaslhs = constp.tile([1, B], F16)
            nc.sync.dma_start(biaslhs[:, :], blhs_d[:, :])

            L1 = l1p.tile([128, G, J], F16)  # resident votes, 16 MB
            logits = logitsp.tile([128, G, O], F16)

            # ================= P0a: pre1 partial only (W stream 1) ==========
            wscope = tc.tile_pool(name="wst", bufs=2)
            wp = wscope.__enter__()
            pre_ps = preps.tile([B, J], F32, tag="pre")
            for gp in range(G // 4):
                wt = wp.tile([128, 4, J], F16, tag="wt")
                nc.sync.dma_start(wt[:, :, :], w_d[gp, :, :, :])
                for gi in range(4):
                    g = 4 * gp + gi
                    for c in range(4):
                        cs = slice(c * 512, (c + 1) * 512)
                        # pre1 partial: uniform-route sum (xdn pre-scaled 1/64)
                        nc.tensor.matmul(
                            pre_ps[:, cs],
                            xdn[:, g, :],
                            wt[:, gi, cs],
                            start=(g == 0),
                            stop=False,
                        )
            # fold bias/NCORES into the partial so squash skips the bias add
            for c in range(4):
                cs = slice(c * 512, (c + 1) * 512)
                nc.tensor.matmul(
                    pre_ps[:, cs],
                    biaslhs[:, :],
                    biasrow[:, cs],
                    start=False,
                    stop=True,
                )

            # ================= P0b: votes production (W stream 2) ===========
            def produce_votes():
                for gp in range(G // 4):
                    wt = wp.tile([128, 4, J], F16, tag="wt")
                    nc.sync.dma_start(wt[:, :, :], w_d[gp, :, :, :])
                    for gi in range(4):
                        g = 4 * gp + gi
                        for c in range(4):
                            cs = slice(c * 512, (c + 1) * 512)
                            pm = mmps.tile([128, 512], F32, tag="pm")
                            nc.tensor.matmul(
                                pm[:, :], xbd[:, g, :], wt[:, gi, cs],
                                start=True, stop=True,
                            )
                            if c % 2 == 0:
                                nc.vector.tensor_copy(L1[:, g, cs], pm[:, :])
                            else:
                                nc.scalar.copy(L1[:, g, cs], pm[:, :])

            # ================= iteration boundaries =================
            actbx = actbxp.tile([128, J], F16)

            def squash_to_actbx(cc_out_t):
                """DMA AR result in, + bias, squash, then broadcast to 128 partitions."""
                pre_sb = smallp.tile([B, J], F32, tag="pre_sb")
                nc.sync.dma_start(pre_sb[:, :], cc_out_t[:, :])
                sq = smallp.tile([B, J], F32, tag="preout")
                nc.scalar.activation(sq[:, :], pre_sb[:, :], ACTFN.Square)
                ns = smallp.tile([B, O], F32, tag="ns")
                nc.vector.tensor_reduce(
                    ns[:, :],
                    sq[:, :].rearrange("p (a o) -> p o a", a=A),
                    axis=AX.X,
                    op=ALU.add,
                )
                # sqrt(ns) = exp(0.5*ln(ns)): stays in the natural_log_exp
                # ACT table set that softmax Exp uses (no ~2.7us set reloads),
                # and is more accurate than the Sqrt spline (65536-ULP budget).
                rt = smallp.tile([B, O], F32, tag="rt")
                nc.scalar.activation(rt[:, :], ns[:, :], ACTFN.Ln)
                rci = smallp.tile([B, O], F32, tag="rci")
                nc.scalar.activation(rci[:, :], rt[:, :], ACTFN.Exp, scale=0.5)
                den = smallp.tile([B, O], F32, tag="den")
                nc.vector.tensor_scalar_add(den[:, :], ns[:, :], 1.0)
                nc.vector.reciprocal(den[:, :], den[:, :])
                s = smallp.tile([B, O], F32, tag="s")
                nc.vector.tensor_mul(s[:, :], den[:, :], rci[:, :])
                act16 = smallp.tile([B, J], F16, tag="act16")
                nc.vector.tensor_mul(
                    act16[:, :].rearrange("p (a o) -> p a o", a=A),
                    pre_sb[:, :].rearrange("p (a o) -> p a o", a=A),
                    s[:, :].rearrange("p (u o) -> p u o", u=1).broadcast_to((B, A, O)),
                )
                # broadcast act to (b,i)-partition layout via dup matmul
                for c in range(4):
                    cs = slice(c * 512, (c + 1) * 512)
                    pm = mmps.tile([128, 512], F32)
                    nc.tensor.matmul(
                        pm[:, :], dup16[:, :], act16[:, cs], start=True, stop=True
                    )
                    if c % 2 == 0:
                        nc.vector.tensor_copy(actbx[:, cs], pm[:, :])
                    else:
                        nc.scalar.copy(actbx[:, cs], pm[:, :])

            def start_allreduce(t, pre_ps_prev):
                pre_sb_out = smallp.tile([B, J], F32, tag="preout")
                nc.scalar.copy(pre_sb_out[:, :], pre_ps_prev[:, :])
                nc.sync.dma_start(cc_in[t][:, :], pre_sb_out[:, :])
                nc.gpsimd.collective_compute(
                    "AllReduce",
                    ALU.add,
                    replica_groups=rg,
                    ins=[cc_in[t][:, :]],
                    outs=[cc_out[t][:, :]],
                )

            # AR1 overlaps the votes production (no dependency on act1);
            # the W streaming pool closes before iteration scratch pools open.
            start_allreduce(0, pre_ps)
            produce_votes()
            wscope.__exit__(None, None, None)

            itstack = [
                tc.tile_pool(name="dtmp", bufs=3),
                tc.tile_pool(name="s1", bufs=2),
                tc.tile_pool(name="s2", bufs=2),
                tc.tile_pool(name="s3", bufs=1),
                tc.tile_pool(name="s4", bufs=1),
            ]
            dpool, s1p, s2p, s3p, s4p = [p.__enter__() for p in itstack]

            def iteration(t, first_dist):
                """squash(AR result) -> distances+route+next pre partial."""
                squash_to_actbx(cc_out[t])

                pre_ps_next = preps.tile([B, J], F32, tag="pre")

                def dist_part(g):
                    dt = dpool.tile([128, J], F16, tag="dtmp")
                    nc.vector.tensor_mul(dt[:, :], L1[:, g, :], actbx[:, :])
                    s1 = s1p.tile([128, 1024], F16)
                    nc.vector.tensor_add(s1[:, :], dt[:, :1024], dt[:, 1024:])
                    s2 = s2p.tile([128, 512], F16)
                    nc.vector.tensor_add(s2[:, :], s1[:, :512], s1[:, 512:])
                    s3 = s3p.tile([128, 256], F16)
                    nc.vector.tensor_add(s3[:, :], s2[:, :256], s2[:, 256:])
                    s4 = s4p.tile([128, 128], F16)
                    nc.vector.tensor_add(s4[:, :], s3[:, :128], s3[:, 128:])
                    if first_dist:
                        nc.vector.tensor_add(
                            logits[:, g, :], s4[:, :64], s4[:, 64:]
                        )
                    else:
                        s5 = s4p.tile([128, 64], F16, tag="s5")
                        nc.vector.tensor_add(s5[:, :], s4[:, :64], s4[:, 64:])
                        nc.vector.tensor_add(logits[:, g, :], logits[:, g, :], s5[:, :])

                def route_part(g):
                    r = _device_softmax_route(nc, softmax_pools, logits[:, g, :], g)
                    wv = dpool.tile([128, J], F16, tag="wv")
                    nc.vector.tensor_mul(
                        wv[:, :].rearrange("p (a o) -> p a o", a=A),
                        L1[:, g, :].rearrange("p (a o) -> p a o", a=A),
                        r[:, :].rearrange("p (u o) -> p u o", u=1).broadcast_to(
                            (128, A, O)
                        ),
                    )
                    for c in range(4):
                        cs = slice(c * 512, (c + 1) * 512)
                        nc.tensor.matmul(
                            pre_ps_next[:, cs],
                            onesbd[:, :],
                            wv[:, cs],
                            start=(g == 0),
                            stop=False,
                        )

                # 1-group software pipeline: softmax/wv of g-1 issues while
                # DVE streams g's distance chain, hiding the ACT round trips.
                for g in range(G):
                    dist_part(g)
                    if g >= 1:
                        route_part(g - 1)
                route_part(G - 1)
                for c in range(4):
                    cs = slice(c * 512, (c + 1) * 512)
                    nc.tensor.matmul(
                        pre_ps_next[:, cs],
                        biaslhs[:, :],
                        biasrow[:, cs],
                        start=False,
                        stop=True,
                    )
                return pre_ps_next

            pre2_ps = iteration(0, first_dist=True)
            start_allreduce(1, pre2_ps)
            pre3_ps = iteration(1, first_dist=False)

            out_sb = smallp.tile([B, J], F32, tag="preout")
            nc.scalar.copy(out_sb[:, :], pre3_ps[:, :])
            nc.sync.dma_start(out_d[:, :], out_sb[:, :])
            for p in reversed(itstack):
                p.__exit__(None, None, None)

    nc.finalize()
    return nc


_NC_CACHE = None


def _get_nc():
    global _NC_CACHE
    if _NC_CACHE is None:
        _NC_CACHE = build_nc()
    return _NC_CACHE


def prepare_inputs(x, weights):
    """Host-side sharding and layout prep. Returns list of per-core input dicts."""
    x = np.asarray(x, np.float32)[..., 0]  # (B, I, K)
    W = np.asarray(weights, np.float32)  # (I, K, J) with j = o*A + a

    # j' = a*64 + o  (a outer, o inner)
    Wp = (
        W.reshape(I, K, O, A).transpose(0, 1, 3, 2).reshape(I, K, J).astype(np.float16)
    )

    onesbd = np.zeros((128, B), np.float16)
    dup16 = np.zeros((B, 128), np.float16)
    for b in range(B):
        onesbd[b * 8 : (b + 1) * 8, b] = 1.0
        dup16[b, b * 8 : (b + 1) * 8] = 1.0

    in_maps = []
    for c in range(NCORES):
        xs = x[:, c * ILOC : (c + 1) * ILOC, :]  # (B, 256, K)
        # w: (G, 128, J) with row p = isub*16 + k
        wc = Wp[c * ILOC : (c + 1) * ILOC].reshape(G, 8 * K, J)
        wc = wc.reshape(G // 4, 4, 128, J).transpose(0, 2, 1, 3)
        # xbd: (128, G, 128): [isub*16+k, g, b*8+isub'] = x[b, 8g+isub, k] iff isub==isub'
        xbd = np.zeros((128, G, 128), np.float16)
        xdn = np.zeros((128, G, B), np.float16)
        xg = xs.reshape(B, G, 8, K)  # b, g, isub, k
        for isub in range(8):
            # rows isub*16 : isub*16+16, cols b*8+isub
            xbd[isub * K : (isub + 1) * K, :, isub::8] = xg[:, :, isub, :].transpose(
                2, 1, 0
            )
            xdn[isub * K : (isub + 1) * K, :, :] = (
                xg[:, :, isub, :].transpose(2, 1, 0) / 64.0
            )
        in_maps.append(
            {
                "w": np.ascontiguousarray(wc),
                "xbd": xbd,
                "xdn": xdn,
                "onesbd": onesbd,
                "dup16": dup16,
                "biasrow": np.zeros((1, J), np.float16),  # placeholder
                "biaslhs": np.full((1, B), 1.0 / NCORES, np.float16),
            }
        )
    return in_maps


def kernel(x, weights, bias):
    bias = np.asarray(bias, np.float32)  # (O, A)
    in_maps = prepare_inputs(x, weights)
    biasb = np.broadcast_to(
        bias.T.reshape(1, J), (B, J)
    ).copy()  # j' = a*64+o -> bias.T is (A, O)
    for m in in_maps:
        m["biasrow"] = biasb[:1].astype(np.float16)

    nc = _get_nc()
    res = run_bass_kernel_spmd(nc, in_maps, core_ids=list(range(NCORES)))
    partials = [res.results[c]["partial"] for c in range(NCORES)]

    total = np.sum(np.stack(partials, 0), axis=0, dtype=np.float64).astype(np.float32)
    pre3 = total.reshape(B, A, O)
    act = _squash_host(pre3)  # (B, A, O)
    return np.ascontiguousarray(act.transpose(0, 2, 1))  # (B, O, A)
